# revision 4
# baseline (speedup 1.0000x reference)
"""Trainium2 Bass kernel v2 for nn_AttentionNet (encoder + 1-step decoder + pointer).

Data-parallel over batch: 4 batches/core x 8 cores. Feature-major on-chip
layout [feature, token]. Key differences vs v1:
  - src loaded NATURALLY (contiguous DMA) then PE-transposed (v1 did a
    4-byte-gather transposing DMA = 524K tiny descriptors).
  - enc_mask loaded via gpsimd cast-DMA int32->bf16 with accum-subtract into
    a ones-initialized tile (w = 1-m for free), then transposed to k-major
    via XBAR dma transposes (v1 burned PE on 256 mask transposes).
  - scores via 4 concurrent K=32 row-group matmuls (tile_position) on
    natural per-head Q/K slices - no query replication (v1 streamed 4x the
    columns at full contraction and spent DVE on building qhat).
  - exp+mask in two lanes: ACT lane (exp then bf16 mask-mult) and DVE lane
    (fused Schraudolph fast-exp + mask in one scalar_tensor_tensor:
    int16((S + B16) * w) bit-viewed as bf16).
  - vacc/dacc via 4 concurrent col-group matmuls (tile_position).
  - reciprocal_approx_fast instead of iterative reciprocal.
  - residual adds folded into PE accumulation (identity matmul).
"""

import math

import numpy as np

import concourse.bacc as bacc
import concourse.bass as bass
import concourse.tile as tile
from concourse import mybir
from concourse.bass_utils import run_bass_kernel_spmd

F32 = mybir.dt.float32
BF16 = mybir.dt.bfloat16
I16 = mybir.dt.int16
I32 = mybir.dt.int32
AF = mybir.ActivationFunctionType
OP = mybir.AluOpType

E, H, D, FF = 128, 4, 32, 512
HD = H * D
B, T, Q = 32, 1024, 1
NCORES = 8
BPC = B // NCORES
NTK = T // 128
TQC = 256
NTQ = T // TQC

A16 = 2.0 ** 7 / math.log(2.0)      # Schraudolph scale for bf16 bit pattern
B16 = 127.0 * 2 ** 7 - 5.5          # Schraudolph bias (offset tuned)
SC = 1.0 / math.sqrt(D)

# per k-tile: 1 = ACT exp lane, 0 = DVE fused fast-exp lane
ACT_LANE = [1, 0, 1, 0, 1, 0, 1, 0]

WEIGHT_NAMES = [
    "enc_wq", "enc_wk", "enc_wv", "enc_wo", "enc_ln1_g", "enc_ln1_b",
    "enc_ln2_g", "enc_ln2_b", "enc_ffn_w1", "enc_ffn_b1", "enc_ffn_w2",
    "enc_ffn_b2",
    "dec_wq", "dec_wk", "dec_wv", "dec_wo", "dec_ln1_g", "dec_ln1_b",
    "dec_ln2_g", "dec_ln2_b", "dec_ffn_w1", "dec_ffn_b1", "dec_ffn_w2",
    "dec_ffn_b2",
    "ptr_wq", "ptr_wk",
]


def _emit(nc, tc, tens, dbg, ctx):
    singles = ctx.enter_context(tc.tile_pool(name="singles", bufs=1))
    big = ctx.enter_context(tc.tile_pool(name="big", bufs=1))
    scratch = ctx.enter_context(tc.tile_pool(name="scratch", bufs=2))
    sc1 = ctx.enter_context(tc.tile_pool(name="sc1", bufs=1))
    psum = ctx.enter_context(tc.tile_pool(name="psum", bufs=1, space="PSUM"))

    cnt = [0]

    def ps_big(bufs=2):   # [128, 1024] f32, 2 banks x bufs
        cnt[0] += 1
        return psum.tile([128, 1024], F32, tag="big", name=f"psB{cnt[0]}", bufs=bufs)

    def ps_med(bufs=2):   # [128, 512] f32, 1 bank x bufs
        cnt[0] += 1
        return psum.tile([128, 512], F32, tag="med", name=f"psM{cnt[0]}", bufs=bufs)

    def ps_tp():          # [128, 512] bf16 transpose target, 1 bank
        cnt[0] += 1
        return psum.tile([128, 512], BF16, tag="xtp", name=f"psT{cnt[0]}", bufs=1)

    def ps_vd():          # [128, 512] f32: vacc in [:, :256], dacc in [:, 256:]
        cnt[0] += 1
        return psum.tile([128, 512], F32, tag="vd", name=f"psVD{cnt[0]}", bufs=1)

    # ---------------- weights ----------------
    def load_w(shape, nm, in_ap, keep=True):
        pool = singles if keep else scratch
        tl = pool.tile(shape, F32, tag=nm if keep else "wstage",
                       name=nm, bufs=1 if keep else 2)
        nc.sync.dma_start(out=tl[:], in_=in_ap)
        return tl

    ln_rows = {}
    for nm in ["enc_ln1_g", "enc_ln1_b", "enc_ln2_g", "enc_ln2_b",
               "dec_ln1_g", "dec_ln1_b", "dec_ln2_g", "dec_ln2_b"]:
        if nm.endswith("ln1_g") or nm.endswith("ln1_b"):
            ln_rows[nm + "_c"] = load_w([E, 1], f"c_{nm}",
                                        tens[nm].ap().rearrange("(e q) -> e q", q=1))
        row = load_w([1, E], f"r_{nm}", tens[nm].ap().rearrange("(q e) -> q e", q=1))
        rb = singles.tile([1, E], BF16, tag=f"rb_{nm}")
        nc.vector.tensor_copy(rb[:], row[:])
        ln_rows[nm] = rb

    wsc = {}
    for pfx in ("enc", "dec"):
        gc = ln_rows[f"{pfx}_ln1_g_c"]
        for nm, scl in (("wq", A16 * SC if pfx == "enc" else 1.0),
                        ("wk", 1.0), ("wv", 1.0)):
            tl = scratch.tile([E, HD], F32, tag="wstage", name=f"{pfx}_{nm}", bufs=2)
            nc.sync.dma_start(out=tl[:].rearrange("e (h d) -> e h d", h=H),
                              in_=tens[f"{pfx}_{nm}"].ap().rearrange("h e d -> e h d"))
            tb = singles.tile([E, HD], BF16, tag=f"{pfx}_{nm}b")
            if pfx == "enc":
                # fold LN1 gain (per input-feature row) and scale
                nc.vector.tensor_scalar(out=tb[:], in0=tl[:], scalar1=gc[:, 0:1],
                                        scalar2=scl, op0=OP.mult, op1=OP.mult)
            else:
                nc.vector.tensor_copy(tb[:], tl[:])
            wsc[f"{pfx}_{nm}"] = tb
        tl = load_w([HD, E], f"{pfx}_wo",
                    tens[f"{pfx}_wo"].ap().rearrange("h d e -> (h d) e"), keep=False)
        tb = singles.tile([HD, E], BF16, tag=f"{pfx}_wob")
        nc.vector.tensor_copy(tb[:], tl[:])
        wsc[f"{pfx}_wo"] = tb

    w1 = {}; w2 = {}; b1t = {}; b2c = {}
    for pfx in ("enc", "dec"):
        tl = load_w([E, FF], f"{pfx}_w1", tens[f"{pfx}_ffn_w1"].ap(), keep=False)
        tb = singles.tile([E, FF], BF16, tag=f"{pfx}_w1b")
        nc.vector.tensor_copy(tb[:], tl[:])
        w1[pfx] = tb
        tf = scratch.tile([128, 4 * E], F32, tag="wstage2", name=f"{pfx}_w2", bufs=2)
        nc.sync.dma_start(out=tf[:].rearrange("p (c e) -> p c e", c=4),
                          in_=tens[f"{pfx}_ffn_w2"].ap().rearrange("(c p) e -> p c e", p=128))
        tb2 = singles.tile([128, 4 * E], BF16, tag=f"{pfx}_w2b")
        nc.vector.tensor_copy(tb2[:], tf[:])
        w2[pfx] = tb2
        b1t[pfx] = load_w([128, 4], f"{pfx}_b1",
                          tens[f"{pfx}_ffn_b1"].ap().rearrange("(c p) -> p c", p=128))
        b2c[pfx] = load_w([128, 1], f"{pfx}_b2",
                          tens[f"{pfx}_ffn_b2"].ap().rearrange("(e q) -> e q", q=1))

    ptrq = load_w([E, E], "ptr_wq", tens["ptr_wq"].ap())
    ptrk = load_w([E, E], "ptr_wk", tens["ptr_wk"].ap())
    ptrkb = singles.tile([E, E], BF16, tag="ptr_wkb")
    nc.vector.tensor_copy(ptrkb[:], ptrk[:])

    # ---------------- constants ----------------
    def const_tile(arr, dt, nm):
        arr = np.asarray(arr)
        if dt == BF16:
            import ml_dtypes
            arr = arr.astype(ml_dtypes.bfloat16)
        elif dt == F32:
            arr = arr.astype(np.float32)
        else:
            arr = arr.astype(np.int32)
        h = nc.inline_tensor(arr, name=f"c_{nm}")
        tl = singles.tile(list(arr.shape), dt, tag=f"c_{nm}", name=f"ct_{nm}")
        nc.sync.dma_start(out=tl[:], in_=h.ap())
        return tl

    ident = const_tile(np.eye(128), BF16, "ident")
    ones_col = const_tile(np.ones((128, 1)), BF16, "ones_col")
    ones_row = const_tile(np.ones((1, 512)), BF16, "ones_row")
    vc = np.ones((1, 128)); vc[0, ::32] = 0.0
    vcomp_row = const_tile(vc, BF16, "vcomp_row")
    a = np.zeros((128, 128))
    for hh in range(H):
        a[32 * hh, 32 * hh:32 * (hh + 1)] = 1.0
    e4sel = const_tile(a, F32, "e4sel")
    a = np.zeros((128, 8 * 36))
    for b in range(4):
        a[:, 36 * b + b] = 1.0
        a[:, 36 * (4 + b) + 32 + b] = 1.0
    ind8 = const_tile(a, BF16, "ind8")
    a = np.zeros((128, 66))
    a[:, 0] = 1.0
    a[:, 33 + 32] = 1.0
    ind2 = const_tile(a, BF16, "ind2")
    a = np.zeros((1, 16))
    for b in range(BPC):
        a[0, 4 * b + b] = 1.0
    eb4 = const_tile(a, BF16, "eb4")
    eps4 = const_tile(np.full((4, 1), 1e-5), F32, "eps4")
    eps1 = const_tile(np.full((1, 1), 1e-5), F32, "eps1")
    a = np.zeros((32, 4))
    for i in range(32):
        a[i, i % 4] = 1.0
    p32 = const_tile(a, F32, "p32")
    a = np.zeros((4, 16))
    for b in range(BPC):
        a[b, 4 * b:4 * (b + 1)] = 1.0
    dsel = const_tile(a, BF16, "dsel")
    a = np.zeros((4, 128))
    for hh in range(H):
        a[hh, 32 * hh:32 * (hh + 1)] = 1.0
    e4t = const_tile(a, F32, "e4t")
    ones_g = const_tile(np.ones((1, E)), BF16, "ones_g")
    zero_b = const_tile(np.zeros((1, E)), BF16, "zero_b")

    # LN1-beta folds: qb/kb per-hd biases added at Q/K drain; vb via rank-1 MM
    bc = ln_rows["enc_ln1_b_c"]
    bcb = singles.tile([E, 1], BF16, tag="enc_betab")
    nc.vector.tensor_copy(bcb[:], bc[:])
    bt_ps = ps_med()
    for i, nm in enumerate(("wq", "wk", "wv")):
        nc.tensor.matmul(bt_ps[:, i:i + 1], wsc[f"enc_{nm}"][:], bcb[:],
                         start=True, stop=True)
    bt = singles.tile([128, 4], F32, tag="enc_bt_s")
    nc.vector.tensor_copy(bt[:], bt_ps[:, 0:4])
    btb = singles.tile([128, 4], BF16, tag="enc_bt_b")
    nc.vector.tensor_copy(btb[:], bt_ps[:, 0:4])
    vrow_ps = ps_tp()
    nc.tensor.matmul(vrow_ps[0:1, 0:128], btb[:, 2:3], ident[:],
                     start=True, stop=True, is_transpose=True)
    vb_row = singles.tile([1, 128], BF16, tag="vb_row")
    nc.vector.tensor_copy(vb_row[:], vrow_ps[0:1, 0:128])

    # ---------------- src natural load + PE transpose ----------------
    srcT = []
    for b in range(BPC):
        xn = sc1.tile([128, 8 * E], F32, tag="xnat", name=f"xnat{b}", bufs=1)
        nc.sync.dma_start(out=xn[:].rearrange("p (j e) -> p j e", j=8),
                          in_=tens["src"].ap()[b].rearrange("(j p) e -> p j e", p=128))
        xb = sc1.tile([128, 8 * E], BF16, tag="xnb", name=f"xnb{b}", bufs=1)
        nc.vector.tensor_copy(xb[:], xn[:])
        xt = big.tile([E, T], BF16, tag=f"srcT{b}")
        for g in range(2):
            tp = ps_tp()
            for j4 in range(4):
                j = 4 * g + j4
                nc.tensor.matmul(tp[:, 128 * j4:128 * (j4 + 1)],
                                 xb[:, E * j:E * (j + 1)], ident[:],
                                 start=True, stop=True, is_transpose=True)
            nc.vector.tensor_copy(xt[:, 512 * g:512 * (g + 1)], tp[:])
        srcT.append(xt)

    # ---------------- LN group (feature-major) ----------------
    def ln_group(xs, g_row, b_row, nm):
        stats_ps = ps_big()
        nb = len(xs)
        for b, xt in enumerate(xs):
            sq = sc1.tile([128, T], BF16, tag="lnsq", name=f"lnsq_{nm}{b}", bufs=1)
            nc.gpsimd.tensor_tensor(out=sq[:], in0=xt[:], in1=xt[:], op=OP.mult)
            for c in range(2):
                s = slice(512 * c, 512 * (c + 1))
                nc.tensor.matmul(stats_ps[0:36, s], ind8[:, 36 * b:36 * (b + 1)],
                                 xt[:, s], start=(b == 0), stop=False)
                nc.tensor.matmul(stats_ps[0:36, s], ind8[:, 36 * (4 + b):36 * (5 + b)],
                                 sq[:, s], start=False, stop=(b == nb - 1))
        st = scratch.tile([36, T], F32, tag="lnsts", name=f"lnsts_{nm}", bufs=1)
        nc.vector.tensor_copy(st[:], stats_ps[0:36, :])
        m = scratch.tile([4, T], F32, tag="lnm", name=f"lnm_{nm}", bufs=1)
        nc.vector.tensor_scalar_mul(m[0:nb], st[0:nb], 1.0 / E)
        var = scratch.tile([4, T], F32, tag="lnvar", name=f"lnvar_{nm}", bufs=1)
        nc.vector.tensor_scalar_mul(var[0:nb], st[32:32 + nb], 1.0 / E)
        msq = scratch.tile([4, T], F32, tag="lns2", name=f"lns2_{nm}", bufs=1)
        nc.gpsimd.tensor_tensor(out=msq[0:nb], in0=m[0:nb], in1=m[0:nb], op=OP.mult)
        nc.vector.tensor_sub(var[0:nb], var[0:nb], msq[0:nb])
        nc.scalar.activation(out=var[0:nb], in_=var[0:nb], func=AF.Ln,
                             bias=eps4[0:nb, 0:1])
        rs = scratch.tile([4, T], BF16, tag="lnrs", name=f"lnrs_{nm}", bufs=1)
        nc.scalar.activation(out=rs[0:nb], in_=var[0:nb], func=AF.Exp, scale=-0.5)
        nmrs = scratch.tile([4, T], BF16, tag="lnnm", name=f"lnnm_{nm}", bufs=1)
        nc.vector.scalar_tensor_tensor(out=nmrs[0:nb], in0=m[0:nb], scalar=-1.0,
                                       in1=rs[0:nb], op0=OP.mult, op1=OP.mult)
        gsel_ps = ps_med()
        for b in range(nb):
            nc.tensor.matmul(gsel_ps[0:4, E * b:E * (b + 1)],
                             eb4[:, 4 * b:4 * (b + 1)], g_row[:], start=True, stop=True)
        gsel = scratch.tile([4, 4 * E], BF16, tag="lngs", name=f"lngs_{nm}", bufs=1)
        nc.vector.tensor_copy(gsel[:], gsel_ps[0:4, 0:4 * E])

        def apply(b, ot):
            xt = xs[b]
            for c in range(2):
                s = slice(512 * c, 512 * (c + 1))
                a_ps = ps_med()
                nc.tensor.matmul(a_ps[:], gsel[:, E * b:E * (b + 1)], rs[0:4, s],
                                 start=True, stop=True)
                b_ps = ps_med()
                nc.tensor.matmul(b_ps[:], gsel[:, E * b:E * (b + 1)], nmrs[0:4, s],
                                 start=True, stop=False)
                nc.tensor.matmul(b_ps[:], b_row[:], ones_row[:], start=False, stop=True)
                tmp = scratch.tile([128, 512], F32, tag="lntmp")
                nc.vector.scalar_tensor_tensor(out=tmp[:], in0=xt[:, s], scalar=1.0,
                                               in1=a_ps[:], op0=OP.bypass, op1=OP.mult)
                nc.vector.scalar_tensor_tensor(out=ot[:, s], in0=tmp[:], scalar=1.0,
                                               in1=b_ps[:], op0=OP.bypass, op1=OP.add)
            return ot
        return apply

    # ---------------- encoder ----------------
    with nc.named_scope("enc_ln1"):
        apply_ln1 = ln_group(srcT, ones_g, zero_b, "l1")

    qhats = []
    for i in range(2):
        qh = big.tile([HD, H * T], BF16, tag=f"qhat{i}")
        nc.vector.memset(qh[:], 0.0)
        qhats.append(qh)
    h1T = []
    with nc.named_scope("enc_attn"):
        for b in range(BPC):
            # mask pipeline for this batch: natural int32 load -> Pool
            # convert w=1-m (bf16) -> PE identity-transposes -> ACT drains
            wn = sc1.tile([128, NTK * T], BF16, tag="wnat", name=f"wnat{b}", bufs=1)
            for j in range(NTK):
                for hf in range(2):
                    mi = sc1.tile([128, 512], I32, tag="mi", name="mi", bufs=2)
                    nc.sync.dma_start(out=mi[:],
                                      in_=tens["enc_mask"].ap()[b][128 * j:128 * (j + 1),
                                                                   512 * hf:512 * (hf + 1)])
                    nc.gpsimd.tensor_scalar(out=wn[:, T * j + 512 * hf:T * j + 512 * (hf + 1)],
                                            in0=mi[:], scalar1=-1.0, scalar2=1.0,
                                            op0=OP.mult, op1=OP.add)
            wt = sc1.tile([128, NTK * T], BF16, tag="wtt", name=f"wtt{b}", bufs=1)
            for k in range(NTK):
                for g in range(2):
                    tp = ps_tp()
                    for j4 in range(4):
                        j = 4 * g + j4
                        nc.tensor.matmul(tp[:, 128 * j4:128 * (j4 + 1)],
                                         wn[:, T * j + 128 * k:T * j + 128 * (k + 1)],
                                         ident[:], start=True, stop=True,
                                         is_transpose=True)
                    nc.scalar.activation(out=wt[:, T * k + 512 * g:T * k + 512 * (g + 1)],
                                         in_=tp[:], func=AF.Copy)

            yt = scratch.tile([E, T], BF16, tag="yT", name=f"yT{b}", bufs=1)
            apply_ln1(b, yt)
            kt = scratch.tile([HD, T], BF16, tag="KT", bufs=1, name=f"KT{b}")
            for c in range(2):
                s = slice(512 * c, 512 * (c + 1))
                qp = ps_med()
                nc.tensor.matmul(qp[:], wsc["enc_wq"][:], yt[:, s], start=True, stop=True)
                qtc = scratch.tile([HD, 512], BF16, tag="qtc")
                nc.scalar.activation(out=qtc[:], in_=qp[:], func=AF.Identity,
                                     bias=bt[:, 0:1])
                for hh in range(H):
                    dst = qhats[b % 2][32 * hh:32 * (hh + 1), :].rearrange(
                        "p (c2 h2 t) -> p c2 h2 t", c2=NTQ, h2=H)[:, 2 * c:2 * c + 2, hh, :]
                    src2 = qtc[32 * hh:32 * (hh + 1), :].rearrange(
                        "p (c2 t) -> p c2 t", c2=2)
                    nc.vector.tensor_copy(dst, src2)
                kp = ps_med()
                nc.tensor.matmul(kp[:], wsc["enc_wk"][:], yt[:, s], start=True, stop=True)
                nc.scalar.activation(out=kt[:, s], in_=kp[:], func=AF.Identity,
                                     bias=bt[:, 1:2])
            vn = scratch.tile([128, NTK * HD], BF16, tag="Vn", bufs=1, name=f"Vn{b}")
            vps = ps_big()
            for c in range(NTK):
                nc.tensor.matmul(vps[:, HD * c:HD * (c + 1)],
                                 yt[:, 128 * c:128 * (c + 1)], wsc["enc_wv"][:],
                                 start=True, stop=False)
                nc.tensor.matmul(vps[:, HD * c:HD * (c + 1)],
                                 ones_row[0:1, 0:128], vb_row[:],
                                 start=False, stop=True)
            nc.vector.tensor_copy(vn[:], vps[:])
            if dbg is not None:
                nc.sync.dma_start(out=dbg["xlnT"].ap()[b], in_=yt[:])

            h1 = big.tile([E, T], BF16, tag=f"h1T{b}")
            for tq in range(NTQ):
                qs = slice(TQC * tq, TQC * (tq + 1))
                vd = ps_vd()
                vacc = vd[:, 0:TQC]
                dacc = vd[:, TQC:2 * TQC]
                nc.tensor.matmul(dacc, vcomp_row[:], ones_row[:, :TQC],
                                 start=True, stop=False)
                S_list = [None] * NTK

                def emit_scores(kk):
                    Sp = ps_big()
                    for half in range(2):
                        fs = slice(512 * half, 512 * (half + 1))
                        nc.tensor.matmul(Sp[:, fs], kt[:, 128 * kk:128 * (kk + 1)],
                                         qhats[b % 2][:, H * TQC * tq + 512 * half:
                                              H * TQC * tq + 512 * (half + 1)],
                                         start=True, stop=True)
                    S_list[kk] = Sp

                emit_scores(0)
                for k in range(NTK):
                    if k + 1 < NTK:
                        emit_scores(k + 1)
                    S = S_list[k]
                    ws = wt[:, T * k + TQC * tq:T * k + TQC * (tq + 1)]
                    eu = sc1.tile([128, H * TQC], I16, tag="eu", name="eu", bufs=2)
                    eub = eu[:].bitcast(BF16)
                    if ACT_LANE[k]:
                        ee = sc1.tile([128, H * TQC], BF16, tag="ee", name="ee", bufs=1)
                        nc.scalar.activation(out=ee[:], in_=S[:], func=AF.Exp,
                                             scale=1.0 / A16)
                        for hh in range(H):
                            hs = slice(TQC * hh, TQC * (hh + 1))
                            nc.vector.tensor_tensor(out=eub[:, hs], in0=ee[:, hs],
                                                    in1=ws, op=OP.mult)
                    else:
                        w_rep = bass.AP(tensor=ws.tensor, offset=ws.offset,
                                        ap=[ws.ap[0], [0, H], ws.ap[1]])
                        nc.vector.scalar_tensor_tensor(
                            out=eu[:].rearrange("p (h t) -> p h t", h=H),
                            in0=S[:].rearrange("p (h t) -> p h t", h=H),
                            scalar=B16, in1=w_rep, op0=OP.add, op1=OP.mult)
                    for hh in range(H):
                        hs = slice(TQC * hh, TQC * (hh + 1))
                        nc.tensor.matmul(vd[32 * hh:32 * (hh + 1), 0:TQC],
                                         vn[:, HD * k + 32 * hh:HD * k + 32 * (hh + 1)],
                                         eub[:, hs], start=(k == 0), stop=(k == NTK - 1),
                                         tile_position=(0, 32 * hh))
                    for hh in range(H):
                        hs = slice(TQC * hh, TQC * (hh + 1))
                        nc.tensor.matmul(vd[32 * hh:32 * hh + 1, TQC:2 * TQC],
                                         ones_col[:], eub[:, hs],
                                         start=False, stop=(k == NTK - 1 and hh == H - 1),
                                         tile_position=(0, 32 * hh))
                recip = scratch.tile([128, TQC], F32, tag="recip")
                nc.vector.reciprocal_approx_fast(out=recip[:], in_=dacc)
                rb_med = ps_med()
                nc.tensor.matmul(rb_med[:, 0:TQC], e4sel[:], recip[:], start=True, stop=True)
                rb = scratch.tile([128, TQC], F32, tag="rbs")
                nc.vector.tensor_copy(rb[:], rb_med[:, 0:TQC])
                hn = scratch.tile([128, TQC], BF16, tag="hn")
                nc.vector.tensor_mul(hn[:], vacc, rb[:])
                at_med = ps_med()
                nc.tensor.matmul(at_med[:, 0:TQC], wsc["enc_wo"][:], hn[:],
                                 start=True, stop=False)
                nc.tensor.matmul(at_med[:, 0:TQC], ident[:], srcT[b][:, qs],
                                 start=False, stop=True)
                nc.scalar.activation(out=h1[:, qs], in_=at_med[:, 0:TQC], func=AF.Copy)
            h1T.append(h1)
            if dbg is not None:
                nc.sync.dma_start(out=dbg["h1T"].ap()[b], in_=h1[:])

    # ---------------- LN2 + FFN ----------------
    with nc.named_scope("enc_ffn"):
        apply_ln2 = ln_group(h1T, ln_rows["enc_ln2_g"], ln_rows["enc_ln2_b"], "l2")
        memT = []
        for b in range(BPC):
            y2 = scratch.tile([E, T], BF16, tag="y2T", name=f"y2T{b}", bufs=1)
            apply_ln2(b, y2)
            mt = big.tile([E, T], BF16, tag=f"memT{b}")
            act1 = scratch.tile([128, 4 * T], BF16, tag="act1", bufs=1)
            for fc in range(4):
                for c in range(2):
                    s = slice(512 * c, 512 * (c + 1))
                    pp = ps_med()
                    nc.tensor.matmul(pp[:], w1["enc"][:, 128 * fc:128 * (fc + 1)],
                                     y2[:, s], start=True, stop=True)
                    nc.scalar.activation(out=act1[:, T * fc + 512 * c:T * fc + 512 * (c + 1)],
                                         in_=pp[:], func=AF.Relu,
                                         bias=b1t["enc"][:, fc:fc + 1], scale=1.0)
            for c in range(2):
                s = slice(512 * c, 512 * (c + 1))
                pp = ps_med()
                for fc in range(4):
                    nc.tensor.matmul(pp[:], w2["enc"][:, 128 * fc:128 * (fc + 1)],
                                     act1[:, T * fc + 512 * c:T * fc + 512 * (c + 1)],
                                     start=(fc == 0), stop=False)
                nc.tensor.matmul(pp[:], ident[:], h1T[b][:, s], start=False, stop=True)
                nc.scalar.activation(out=mt[:, s], in_=pp[:], func=AF.Identity,
                                     bias=b2c["enc"][:, 0:1])
            memT.append(mt)
            if dbg is not None:
                nc.sync.dma_start(out=dbg["memT"].ap()[b], in_=mt[:])

    # ---------------- decoder ----------------
    with nc.named_scope("decoder"):
        tgtT = singles.tile([E, BPC], F32, tag="tgtT")
        nc.sync.dma_start(out=tgtT[:], in_=tens["tgt"].ap().rearrange("b q e -> e (b q)"))
        dmi = scratch.tile([BPC, T], I32, tag="dmi", bufs=1)
        nc.sync.dma_start(out=dmi[:], in_=tens["dec_mask"].ap().rearrange("b q t -> (b q) t"))
        dmf = singles.tile([BPC, T], BF16, tag="dmf")
        nc.vector.tensor_scalar_mul(dmf[:], dmi[:], -10000.0)
        dmf32 = singles.tile([BPC, T], F32, tag="dmf32")
        nc.vector.tensor_scalar_mul(dmf32[:], dmi[:], -10000.0)
        dwf = singles.tile([BPC, T], F32, tag="dwf")
        nc.vector.tensor_scalar(out=dwf[:], in0=dmi[:], scalar1=-1.0, scalar2=1.0,
                                op0=OP.mult, op1=OP.add)

        apply_lnm = ln_group(memT, ln_rows["dec_ln1_g"], ln_rows["dec_ln1_b"], "lm")

        def ln_small(x, n, g_row, b_row, tagn):
            xbf = scratch.tile([E, BPC], BF16, tag=f"dxb{tagn}")
            nc.vector.tensor_copy(xbf[:, :n], x[:, :n])
            sq = scratch.tile([E, BPC], BF16, tag=f"dsq{tagn}")
            nc.gpsimd.tensor_tensor(out=sq[:, :n], in0=xbf[:, :n], in1=xbf[:, :n],
                                    op=OP.mult)
            st_ps = ps_med()
            nc.tensor.matmul(st_ps[0:33, :n], ind2[:, 0:33], xbf[:, :n],
                             start=True, stop=False)
            nc.tensor.matmul(st_ps[0:33, :n], ind2[:, 33:66], sq[:, :n],
                             start=False, stop=True)
            st = scratch.tile([33, BPC], F32, tag=f"dss{tagn}")
            nc.vector.tensor_copy(st[:, :n], st_ps[0:33, :n])
            mn = scratch.tile([1, BPC], F32, tag=f"dmn{tagn}")
            nc.vector.tensor_scalar_mul(mn[:, :n], st[0:1, :n], 1.0 / E)
            msq = scratch.tile([1, BPC], F32, tag=f"dmsq{tagn}")
            nc.vector.tensor_mul(msq[:, :n], mn[:, :n], mn[:, :n])
            var = scratch.tile([1, BPC], F32, tag=f"dvar{tagn}")
            nc.vector.tensor_scalar(out=var[:, :n], in0=st[32:33, :n], scalar1=1.0 / E,
                                    scalar2=None, op0=OP.mult)
            nc.vector.tensor_sub(var[:, :n], var[:, :n], msq[:, :n])
            nc.scalar.activation(out=var[:, :n], in_=var[:, :n], func=AF.Ln, bias=eps1[:])
            rs = scratch.tile([1, BPC], BF16, tag=f"drs{tagn}")
            nc.scalar.activation(out=rs[:, :n], in_=var[:, :n], func=AF.Exp, scale=-0.5)
            nmrs = scratch.tile([1, BPC], BF16, tag=f"dnm{tagn}")
            nc.vector.scalar_tensor_tensor(out=nmrs[:, :n], in0=mn[:, :n], scalar=-1.0,
                                           in1=rs[:, :n], op0=OP.mult, op1=OP.mult)
            a_ps = ps_med()
            nc.tensor.matmul(a_ps[:, :n], g_row[:], rs[:, :n], start=True, stop=True)
            b_ps = ps_med()
            nc.tensor.matmul(b_ps[:, :n], g_row[:], nmrs[:, :n], start=True, stop=False)
            nc.tensor.matmul(b_ps[:, :n], b_row[:], ones_row[:, :n], start=False, stop=True)
            tmp = scratch.tile([E, BPC], F32, tag=f"dtmp{tagn}")
            nc.vector.scalar_tensor_tensor(out=tmp[:, :n], in0=x[:, :n], scalar=1.0,
                                           in1=a_ps[:, :n], op0=OP.bypass, op1=OP.mult)
            out = scratch.tile([E, BPC], BF16, tag=f"dout{tagn}")
            nc.vector.scalar_tensor_tensor(out=out[:, :n], in0=tmp[:, :n], scalar=1.0,
                                           in1=b_ps[:, :n], op0=OP.bypass, op1=OP.add)
            return out

        tln = ln_small(tgtT, BPC, ln_rows["dec_ln1_g"], ln_rows["dec_ln1_b"], "t")
        qd_ps = ps_med()
        nc.tensor.matmul(qd_ps[0:HD, 0:BPC], wsc["dec_wq"][:], tln[:, :BPC],
                         start=True, stop=True)
        qdec = scratch.tile([HD, BPC], BF16, tag="qdec")
        nc.vector.tensor_copy(qdec[:], qd_ps[0:HD, 0:BPC])

        h1d = singles.tile([E, BPC], F32, tag="h1d")
        for b in range(BPC):
            mlnb = scratch.tile([E, T], BF16, tag="mln", name=f"mln{b}", bufs=1)
            apply_lnm(b, mlnb)
            kd = scratch.tile([HD, T], BF16, tag="kdec", bufs=1)
            vd = scratch.tile([128, NTK * HD], BF16, tag="vdec", bufs=1)
            for c in range(2):
                s = slice(512 * c, 512 * (c + 1))
                pp = ps_med()
                nc.tensor.matmul(pp[:], wsc["dec_wk"][:], mlnb[:, s], start=True, stop=True)
                nc.vector.tensor_copy(kd[:, s], pp[:])
            vps2 = ps_big()
            for c in range(NTK):
                nc.tensor.matmul(vps2[:, HD * c:HD * (c + 1)],
                                 mlnb[:, 128 * c:128 * (c + 1)], wsc["dec_wv"][:],
                                 start=True, stop=True)
            nc.vector.tensor_copy(vd[:], vps2[:])
            qblk = scratch.tile([HD, 4], BF16, tag="qblk")
            nc.vector.memset(qblk[:], 0.0)
            for hh in range(H):
                nc.vector.tensor_copy(qblk[32 * hh:32 * (hh + 1), hh:hh + 1],
                                      qdec[32 * hh:32 * (hh + 1), b:b + 1])
            ud_ps = ps_med()
            for k in range(NTK):
                cs = slice(4 * k, 4 * (k + 1))
                nc.tensor.matmul(ud_ps[:, cs], kd[:, 128 * k:128 * (k + 1)], qblk[:],
                                 start=True, stop=False)
                nc.tensor.matmul(ud_ps[:, cs], dmf[0:4, 128 * k:128 * (k + 1)],
                                 dsel[:, 4 * b:4 * (b + 1)], start=False, stop=True)
            eud = scratch.tile([128, 4 * NTK], BF16, tag="eud")
            nc.scalar.activation(out=eud[:], in_=ud_ps[:, 0:4 * NTK], func=AF.Exp,
                                 scale=SC)
            d1_ps = ps_med()
            nc.tensor.matmul(d1_ps[0:32, 0:1], eud[:], ones_col[:], start=True, stop=True)
            d1 = scratch.tile([32, 1], F32, tag="d1s")
            nc.vector.tensor_copy(d1[:], d1_ps[0:32, 0:1])
            d4_ps = ps_med()
            nc.tensor.matmul(d4_ps[0:4, 0:1], p32[:], d1[:], start=True, stop=True)
            rc4d = scratch.tile([4, 1], F32, tag="rc4d")
            nc.vector.reciprocal_approx_fast(out=rc4d[:], in_=d4_ps[0:4, 0:1])
            rb_ps = ps_med()
            nc.tensor.matmul(rb_ps[:, 0:1], e4t[:], rc4d[:], start=True, stop=True)
            rbd = scratch.tile([128, 1], F32, tag="rb128s")
            nc.vector.tensor_copy(rbd[:], rb_ps[:, 0:1])
            hd_ps = ps_med()
            for k in range(NTK):
                nc.tensor.matmul(hd_ps[:, 0:4], vd[:, HD * k:HD * (k + 1)],
                                 eud[:, 4 * k:4 * (k + 1)],
                                 start=(k == 0), stop=(k == NTK - 1))
            hdec = scratch.tile([HD, 1], BF16, tag="hdec")
            for hh in range(H):
                nc.vector.tensor_copy(hdec[32 * hh:32 * (hh + 1), 0:1],
                                      hd_ps[32 * hh:32 * (hh + 1), hh:hh + 1])
            nc.vector.tensor_scalar_mul(hdec[:], hdec[:], rbd[:, 0:1])
            ao_ps = ps_med()
            nc.tensor.matmul(ao_ps[:, 0:1], wsc["dec_wo"][:], hdec[:], start=True, stop=True)
            nc.vector.tensor_add(h1d[:, b:b + 1], ao_ps[:, 0:1], tgtT[:, b:b + 1])

        hln2d = ln_small(h1d, BPC, ln_rows["dec_ln2_g"], ln_rows["dec_ln2_b"], "d2")
        dact_ps = ps_med()
        for fc in range(4):
            nc.tensor.matmul(dact_ps[:, 4 * fc:4 * (fc + 1)],
                             w1["dec"][:, 128 * fc:128 * (fc + 1)], hln2d[:, :BPC],
                             start=True, stop=True)
        dact = scratch.tile([128, 16], BF16, tag="dacts")
        for fc in range(4):
            nc.scalar.activation(out=dact[:, 4 * fc:4 * (fc + 1)],
                                 in_=dact_ps[:, 4 * fc:4 * (fc + 1)], func=AF.Relu,
                                 bias=b1t["dec"][:, fc:fc + 1], scale=1.0)
        do_ps = ps_med()
        for fc in range(4):
            nc.tensor.matmul(do_ps[:, 0:BPC], w2["dec"][:, 128 * fc:128 * (fc + 1)],
                             dact[:, 4 * fc:4 * (fc + 1)],
                             start=(fc == 0), stop=(fc == 3))
        decT = singles.tile([E, BPC], F32, tag="decT")
        tmp2 = scratch.tile([E, BPC], F32, tag="dtmp2")
        nc.vector.tensor_scalar_add(tmp2[:], do_ps[:, 0:BPC], b2c["dec"][:, 0:1])
        nc.vector.tensor_add(decT[:], tmp2[:], h1d[:])
        if dbg is not None:
            nc.sync.dma_start(out=dbg["decT"].ap(), in_=decT[:])

    # ---------------- pointer ----------------
    with nc.named_scope("pointer"):
        qp_ps = ps_med()
        nc.tensor.matmul(qp_ps[:, 0:BPC], ptrq[:], decT[:], start=True, stop=True)
        qpi = scratch.tile([E, 4 * BPC], F32, tag="qpi")
        nc.vector.memset(qpi[:], 0.0)
        for b in range(BPC):
            nc.vector.tensor_copy(qpi[:, 5 * b:5 * b + 1], qp_ps[:, b:b + 1])
        up_ps = ps_big()
        for b in range(BPC):
            kp = scratch.tile([E, T], BF16, tag="kdec", name="kps", bufs=1)
            for c in range(2):
                s = slice(512 * c, 512 * (c + 1))
                kpc = ps_med()
                nc.tensor.matmul(kpc[:], ptrkb[:], memT[b][:, s], start=True, stop=True)
                nc.vector.tensor_copy(kp[:, s], kpc[:])
            qpib = scratch.tile([E, 4], BF16, tag="qpib", name=f"qpib{b}", bufs=2)
            nc.vector.tensor_copy(qpib[:], qpi[:, 4 * b:4 * (b + 1)])
            for c in range(2):
                s = slice(512 * c, 512 * (c + 1))
                nc.tensor.matmul(up_ps[0:BPC, s], qpib[:], kp[:, s],
                                 start=(b == 0), stop=(b == BPC - 1))
        e2 = scratch.tile([BPC, T], F32, tag="pe2", bufs=1)
        nc.scalar.activation(out=e2[:], in_=up_ps[0:BPC, :], func=AF.Exp,
                             scale=2.0 / math.sqrt(E))
        nc.vector.tensor_scalar_add(e2[:], e2[:], 1.0)
        rec = scratch.tile([BPC, T], F32, tag="prec", bufs=1)
        nc.vector.reciprocal_approx_fast(out=rec[:], in_=e2[:])
        L = scratch.tile([BPC, T], F32, tag="pL", bufs=1)
        nc.vector.tensor_scalar(out=L[:], in0=rec[:], scalar1=-20.0, scalar2=10.0,
                                op0=OP.mult, op1=OP.add)
        nc.vector.tensor_mul(L[:], L[:], dwf[:])
        nc.vector.tensor_add(L[:], L[:], dmf32[:])
        et = scratch.tile([BPC, T], F32, tag="pet", bufs=1)
        se = scratch.tile([BPC, 1], F32, tag="se")
        nc.scalar.activation(out=et[:], in_=L[:], func=AF.Exp, accum_out=se[:])
        lse = scratch.tile([BPC, 1], F32, tag="lse")
        nc.scalar.activation(out=lse[:], in_=se[:], func=AF.Ln)
        res = scratch.tile([BPC, T], F32, tag="pres", bufs=1)
        nc.vector.tensor_scalar(out=res[:], in0=L[:], scalar1=lse[:, 0:1], scalar2=None,
                                op0=OP.subtract)
        nc.sync.dma_start(out=tens["out"].ap().rearrange("b q t -> (b q) t"), in_=res[:])


def build(debug=False):
    import contextlib
    nc = bacc.Bacc()
    tens = {}
    tens["src"] = nc.dram_tensor("src", [BPC, T, E], F32, kind="ExternalInput")
    tens["tgt"] = nc.dram_tensor("tgt", [BPC, Q, E], F32, kind="ExternalInput")
    tens["enc_mask"] = nc.dram_tensor("enc_mask", [BPC, T, T], I32, kind="ExternalInput")
    tens["dec_mask"] = nc.dram_tensor("dec_mask", [BPC, Q, T], I32, kind="ExternalInput")
    shapes = {
        "wq": [H, E, D], "wk": [H, E, D], "wv": [H, E, D], "wo": [H, D, E],
        "ln1_g": [E], "ln1_b": [E], "ln2_g": [E], "ln2_b": [E],
        "ffn_w1": [E, FF], "ffn_b1": [FF], "ffn_w2": [FF, E], "ffn_b2": [E],
    }
    for pfx in ("enc", "dec"):
        for nm, shp in shapes.items():
            full = f"{pfx}_{nm}"
            tens[full] = nc.dram_tensor(full, shp, F32, kind="ExternalInput")
    tens["ptr_wq"] = nc.dram_tensor("ptr_wq", [E, E], F32, kind="ExternalInput")
    tens["ptr_wk"] = nc.dram_tensor("ptr_wk", [E, E], F32, kind="ExternalInput")
    tens["out"] = nc.dram_tensor("out", [BPC, Q, T], F32, kind="ExternalOutput")

    dbg = None
    if debug:
        dbg = {
            "xlnT": nc.dram_tensor("dbg_xlnT", [BPC, E, T], BF16, kind="ExternalOutput"),
            "h1T": nc.dram_tensor("dbg_h1T", [BPC, E, T], BF16, kind="ExternalOutput"),
            "memT": nc.dram_tensor("dbg_memT", [BPC, E, T], BF16, kind="ExternalOutput"),
            "decT": nc.dram_tensor("dbg_decT", [E, BPC], F32, kind="ExternalOutput"),
        }

    with tile.TileContext(nc) as tc:
        with contextlib.ExitStack() as ctx:
            _emit(nc, tc, tens, dbg, ctx)
    nc.finalize()
    return nc


_built = {}


def _get_nc(debug=False):
    key = bool(debug)
    if key not in _built:
        _built[key] = build(debug=key)
    return _built[key]


def make_in_maps(inputs):
    in_maps = []
    for c in range(NCORES):
        s = slice(BPC * c, BPC * (c + 1))
        m = {
            "src": np.ascontiguousarray(inputs["src"][s]),
            "tgt": np.ascontiguousarray(inputs["tgt"][s]),
            "enc_mask": np.ascontiguousarray(inputs["enc_mask"][s]),
            "dec_mask": np.ascontiguousarray(inputs["dec_mask"][s]),
        }
        for nm in WEIGHT_NAMES:
            m[nm] = np.asarray(inputs[nm])
        in_maps.append(m)
    return in_maps


def kernel(**inputs):
    nc = _get_nc(debug=False)
    in_maps = make_in_maps(inputs)
    res = run_bass_kernel_spmd(nc, in_maps, list(range(NCORES)))
    out = np.concatenate([res.results[c]["out"] for c in range(NCORES)], axis=0)
    return out.astype(np.float32)
